# revision 1
# baseline (speedup 1.0000x reference)
"""MoE routing kernel for Trainium2 (8 NeuronCores, expert-parallel).

out[i] = x[i] + relu(x[i] @ W[e].T + b[e]),  e = cam_pred_ids[i]

Strategy: route tokens by expert on the host (the sharding step), so core e
computes ONLY expert e's tokens with ONLY W[e] (16MB instead of 128MB).
All device-side layouts are pre-transposed on the host so every DMA is
contiguous: the device computes hT[o, n] = sum_k WT[k, o] * xT[k, n] with
K on partitions for both operands, then outT = xT + relu(hT + b) and the
host transposes/scatters back.
"""

import os
import numpy as np

import concourse.bass as bass
from concourse import bacc
import concourse.mybir as mybir
import concourse.tile as tile
from concourse.bass_utils import run_bass_kernel_spmd

NUM_EXPERTS = 8
DIM = 2048
KT = DIM // 128  # 16 k-tiles
OT = DIM // 128  # 16 o-tiles

# matmul operand dtype mode: 'f16' (default: 1 cyc/row, fully-overlapped
# weight loads, end-to-end rel err ~1.1e-4), 'f32r' (fp32 storage,
# TF32-like matmul, 1 cyc/row but exposed fp32 weight loads), 'f32'
# (exact, 4 cyc/row), 'bf16'. The residual add always uses exact fp32 x.
MODE = os.environ.get("BASS_MOE_MODE", "f16")


def _chunks(np_tokens: int) -> list[tuple[int, int]]:
    """Split the free dim into matmul chunks of <=512, each >=256 when
    possible (float32r runs 4x slower below 256 moving columns)."""
    out = []
    pos = 0
    rem = np_tokens
    while rem > 0:
        if rem > 512 + 256:
            take = 512
        elif rem > 512:
            take = (rem + 1) // 2  # two chunks, both >=256
        else:
            take = rem
        out.append((pos, take))
        pos += take
        rem -= take
    return out


def _build_nc(np_tokens: int, mode: str):
    f32 = mybir.dt.float32
    mm_dt = {
        "f32r": mybir.dt.float32r,
        "f32": mybir.dt.float32,
        "f16": mybir.dt.float16,
        "bf16": mybir.dt.bfloat16,
    }[mode]
    sixteen_bit = mode in ("f16", "bf16")

    # For f32/f32r the x input doubles as matmul rhs and residual; walrus
    # requires fp32r matmul operands to be produced as fp32r, so the tiles
    # carry mm_dt and get bitcast to f32 for the residual add (same bits).
    xt_dt = mm_dt if mode in ("f32", "f32r") else f32

    nc = bacc.Bacc()
    wt_d = nc.declare_dram_parameter("wt", [OT, 128, KT, 128], mm_dt, isOutput=False)
    xt_d = nc.declare_dram_parameter("xt", [DIM, np_tokens], xt_dt, isOutput=False)
    if sixteen_bit:
        xtm_d = nc.declare_dram_parameter(
            "xtm", [DIM, np_tokens], mm_dt, isOutput=False
        )
    b_d = nc.declare_dram_parameter("b", [128, OT], f32, isOutput=False)
    out_d = nc.declare_dram_parameter("out", [DIM, np_tokens], f32, isOutput=True)

    chunks = _chunks(np_tokens)
    relu = mybir.ActivationFunctionType.Relu

    with tile.TileContext(nc) as tc:
        with (
            tc.tile_pool(name="xp", bufs=1) as xp,
            tc.tile_pool(name="wp", bufs=4) as wp,
            tc.tile_pool(name="op", bufs=3) as op,
            tc.tile_pool(name="bp", bufs=1) as bp,
            tc.tile_pool(name="pp", bufs=2, space="PSUM") as pp,
        ):
            # First weight tile is DMA'd BEFORE the x loads (split in pieces
            # so the first matmul group can start on kt=0 early): queue
            # semaphore targets accumulate in program order, so anything
            # queued ahead delays the first matmul's wait from clearing.
            # DMA triggers cost ~650ns of serial sequencer time each
            # (DIRECT2D) and each HWDGE ring drains serially, so split by
            # role: sync ring carries the weight stream (+ half the matmul
            # x), scalar ring carries bias + other half + the f32 residual
            # copy + output stores. The PE-gating loads (wt, xmm) never
            # queue behind the residual load.
            wtiles = {}
            wtiles[0] = wp.tile([128, KT, 128], mm_dt, name="wtile", tag="w")
            for q in range(2):
                nc.sync.dma_start(
                    out=wtiles[0][:, q * 8 : (q + 1) * 8, :],
                    in_=wt_d[0, :, q * 8 : (q + 1) * 8, :],
                )

            btile = bp.tile([128, OT], f32, name="btile")
            nc.scalar.dma_start(out=btile, in_=b_d[:, :])

            xt_r = xt_d.rearrange("(t p) n -> p t n", p=128)
            xall = xp.tile([128, KT, np_tokens], xt_dt, name="xall")
            # x loads as 8 slabs of 2 k-tiles each, alternating rings
            slabs = [(s, 2) for s in range(0, KT, 2)]
            if sixteen_bit:
                xtm_r = xtm_d.rearrange("(t p) n -> p t n", p=128)
                xmall = xp.tile([128, KT, np_tokens], mm_dt, name="xmall")
                for q, (s0, w) in enumerate(slabs):
                    eng = nc.sync if q % 2 == 0 else nc.scalar
                    eng.dma_start(
                        out=xmall[:, s0 : s0 + w, :],
                        in_=xtm_r[:, s0 : s0 + w, :],
                    )
                xmm = [xmall[:, kt, :] for kt in range(KT)]
            else:
                for q, (s0, w) in enumerate(slabs):
                    eng = nc.sync if q % 2 == 0 else nc.scalar
                    eng.dma_start(
                        out=xall[:, s0 : s0 + w, :],
                        in_=xt_r[:, s0 : s0 + w, :],
                    )

            # Whole weight stream up front on the sync ring; slot waits
            # (wp bufs) pace it automatically behind the matmul progress.
            for ot in range(1, OT):
                wtiles[ot] = wp.tile([128, KT, 128], mm_dt, name="wtile", tag="w")
                nc.sync.dma_start(out=wtiles[ot], in_=wt_d[ot])

            if sixteen_bit:
                # residual copy: only needed when psum groups close — load
                # late, on the scalar ring
                for q in range(8):
                    nc.scalar.dma_start(
                        out=xall[:, q * 2 : (q + 1) * 2, :],
                        in_=xt_r[:, q * 2 : (q + 1) * 2, :],
                    )
            xall_f32 = xall.bitcast(f32) if mode == "f32r" else xall
            xres = [xall_f32[:, kt, :] for kt in range(KT)]
            if not sixteen_bit:
                xmm = [xall[:, kt, :] for kt in range(KT)]

            for ot in range(OT):
                wtile = wtiles[ot]
                otile = op.tile([128, np_tokens], f32, name="otile", tag="o")
                psums = [
                    pp.tile([128, ch], f32, name=f"ps{ci}", tag=f"ps{ci}")
                    for ci, (_, ch) in enumerate(chunks)
                ]
                for kt in range(KT):
                    lhsT = wtile[:, kt, :]
                    for ci, (n0, ch) in enumerate(chunks):
                        nc.tensor.matmul(
                            psums[ci],
                            lhsT,
                            xmm[kt][:, n0 : n0 + ch],
                            start=(kt == 0),
                            stop=(kt == KT - 1),
                        )
                for ci, (n0, ch) in enumerate(chunks):
                    nc.scalar.activation(
                        otile[:, n0 : n0 + ch],
                        psums[ci],
                        relu,
                        bias=btile[:, ot : ot + 1],
                    )
                    nc.vector.tensor_add(
                        otile[:, n0 : n0 + ch],
                        otile[:, n0 : n0 + ch],
                        xres[ot][:, n0 : n0 + ch],
                    )
                    if ot == OT - 1:
                        # last tile: store per-chunk to overlap the tail
                        nc.scalar.dma_start(
                            out=out_d[ot * 128 : (ot + 1) * 128, n0 : n0 + ch],
                            in_=otile[:, n0 : n0 + ch],
                        )
                if ot < OT - 1:
                    nc.scalar.dma_start(
                        out=out_d[ot * 128 : (ot + 1) * 128, :], in_=otile
                    )
    nc.compile()
    return nc


def kernel(x, cam_pred_ids, W, b, _want_results=False):
    x = np.ascontiguousarray(np.asarray(x), dtype=np.float32)
    W = np.asarray(W, dtype=np.float32)
    b = np.asarray(b, dtype=np.float32)
    ids = np.asarray(cam_pred_ids).astype(np.int64)
    batch = x.shape[0]

    counts = np.bincount(ids, minlength=NUM_EXPERTS)
    order = np.argsort(ids, kind="stable")
    np_tokens = max(512, int(counts.max()))

    # per-expert padded token index lists (pad with token 0; discarded later)
    starts = np.zeros(NUM_EXPERTS + 1, dtype=np.int64)
    np.cumsum(counts, out=starts[1:])
    idx = np.zeros((NUM_EXPERTS, np_tokens), dtype=np.int64)
    for e in range(NUM_EXPERTS):
        idx[e, : counts[e]] = order[starts[e] : starts[e + 1]]

    mode = MODE
    mm_np = {
        "f32r": np.float32,
        "f32": np.float32,
        "f16": np.float16,
        "bf16": None,  # ml_dtypes.bfloat16, resolved lazily
    }[mode]
    if mode == "bf16":
        import ml_dtypes

        mm_np = ml_dtypes.bfloat16
    sixteen_bit = mode in ("f16", "bf16")

    in_maps = []
    for e in range(NUM_EXPERTS):
        xg = x[idx[e]]  # [Np, DIM]
        xt = np.ascontiguousarray(xg.T)  # [DIM, Np]
        # wdev[ot, k, kt, o] = W[e][ot*128+o, kt*128+k]
        wdev = np.ascontiguousarray(
            W[e].reshape(OT, 128, KT, 128).transpose(0, 3, 2, 1), dtype=mm_np
        )
        m = {
            "wt": wdev,
            "xt": xt,
            "b": np.ascontiguousarray(b[e].reshape(OT, 128).T),
        }
        if sixteen_bit:
            m["xtm"] = np.ascontiguousarray(xt, dtype=mm_np)
        in_maps.append(m)

    nc = _build_nc(np_tokens, mode)
    res = run_bass_kernel_spmd(
        nc,
        in_maps,
        core_ids=list(range(NUM_EXPERTS)),
        trace=bool(int(os.environ.get("BASS_MOE_TRACE", "0"))),
    )

    out = np.empty_like(x)
    for e in range(NUM_EXPERTS):
        oute = res.results[e]["out"]  # [DIM, Np]
        valid = idx[e, : counts[e]]
        out[valid] = oute.T[: counts[e]]
    if _want_results:
        return out, res
    return out



# revision 5
# speedup vs baseline: 1.0559x; 1.0559x over previous
"""MoE routing kernel for Trainium2 (8 NeuronCores, expert-parallel).

out[i] = x[i] + relu(x[i] @ W[e].T + b[e]),  e = cam_pred_ids[i]

Strategy: route tokens by expert on the host, so core e computes ONLY
expert e's tokens with ONLY W[e] (8MB in f16 instead of 128MB).  The
device computes hT[o, n] = sum_k WT[k, o] * xT[k, n] with K on
partitions for both operands; everything on device is f16 except PSUM
and the bias (end-to-end rel err ~3e-4).

Loop structure (v2): column chunks of <=512 (one PSUM bank) x ot-blocks
of 4 (so 2 block-groups ping-pong across the 8 PSUM banks).  Within a
group the kt loop is OUTER, so the first matmul only needs the first
k-tiles of x and of weight block 0 -- compute starts ~1us into the
kernel instead of waiting for the full x load.  The whole weight stack
stays resident in SBUF (64KB/partition) and later chunks re-read it
for free; x is loaded once (f16, doubles as matmul rhs and residual).
Output is stored f16 and widened on the host.

Ring split (HWDGE rings exist only on SP/sync and Activation/scalar):
sync ring carries the weight stream (block 0 split in 2-kt pieces to
race the PE) plus the output stores (one per group, starting ~14us in,
after the weight triggers); scalar ring carries x (chunk 0 split in
2-kt pieces) + bias.
"""

import os
import numpy as np

import concourse.bass as bass
from concourse import bacc
import concourse.mybir as mybir
import concourse.tile as tile
from concourse.bass_utils import run_bass_kernel_spmd

MODE = "f16"  # informational; device math is f16 (PSUM f32)

NUM_EXPERTS = 8
DIM = 2048
KT = DIM // 128  # 16 k-tiles
OT = DIM // 128  # 16 o-tiles
OB = 4           # o-tiles per block-group (4 PSUM banks per group)
NBLK = OT // OB  # 4 blocks


def _chunks(np_tokens: int) -> list[tuple[int, int]]:
    """Split the free dim into matmul chunks of <=512 (one PSUM bank),
    each >=232 when possible (below ~230 cols the 97ns LDWEIGHTS shadow
    outruns the matmul and the PE stalls on weight loads)."""
    out = []
    pos = 0
    rem = np_tokens
    while rem > 0:
        if rem > 512 + 232:
            take = 512
        elif rem > 512:
            take = (rem + 1) // 2  # two chunks, both >=232
        else:
            take = rem
        out.append((pos, take))
        pos += take
        rem -= take
    return out


def _build_nc(np_tokens: int):
    f32 = mybir.dt.float32
    f16 = mybir.dt.float16

    nc = bacc.Bacc()
    wt_d = nc.declare_dram_parameter("wt", [NBLK, 128, KT, OB, 128], f16,
                                     isOutput=False)
    xt_d = nc.declare_dram_parameter("xt", [DIM, np_tokens], f16, isOutput=False)
    b_d = nc.declare_dram_parameter("b", [128, OT], f32, isOutput=False)
    out_d = nc.declare_dram_parameter("out", [DIM, np_tokens], f16, isOutput=True)

    chunks = _chunks(np_tokens)
    relu = mybir.ActivationFunctionType.Relu

    xt_r = xt_d.rearrange("(t p) n -> p t n", p=128)
    out_r = out_d.rearrange("(t p) n -> p t n", p=128)

    with tile.TileContext(nc) as tc:
        with (
            tc.tile_pool(name="wp", bufs=1) as wp,
            tc.tile_pool(name="xp", bufs=1) as xp,
            tc.tile_pool(name="bp", bufs=1) as bp,
            tc.tile_pool(name="op", bufs=3) as op,
            tc.tile_pool(name="pp", bufs=2, space="PSUM") as pp,
        ):
            wall = wp.tile([128, NBLK, KT, OB, 128], f16, name="wall")
            xm = xp.tile([128, KT, np_tokens], f16, name="xm")
            btile = bp.tile([128, OT], f32, name="btile")

            # Weight block 0 in 2-kt pieces so the first matmuls only wait
            # for the head of the stream; blocks 1-3 are single transfers
            # (consumed 14+ us in, long after they land).
            for s in range(0, KT, 2):
                nc.sync.dma_start(out=wall[:, 0, s:s + 2],
                                  in_=wt_d[0, :, s:s + 2])
            for blk in range(1, NBLK):
                nc.sync.dma_start(out=wall[:, blk], in_=wt_d[blk])

            # x chunk 0 in 2-kt pieces (same reason); later chunks whole.
            (c0, w0) = chunks[0]
            for s in range(0, KT, 2):
                nc.scalar.dma_start(out=xm[:, s:s + 2, c0:c0 + w0],
                                    in_=xt_r[:, s:s + 2, c0:c0 + w0])
            nc.scalar.dma_start(out=btile, in_=b_d[:, :])
            for (cn, wn) in chunks[1:]:
                nc.scalar.dma_start(out=xm[:, :, cn:cn + wn],
                                    in_=xt_r[:, :, cn:cn + wn])

            for (cn, wn) in chunks:
                for blk in range(NBLK):
                    ps = [
                        pp.tile([128, 512], f32, name="ps", tag=f"ps{oi}")
                        for oi in range(OB)
                    ]
                    otile = op.tile([128, OB, 512], f16, name="otile", tag="o")
                    for kt in range(KT):
                        for oi in range(OB):
                            nc.tensor.matmul(
                                ps[oi][:, :wn],
                                wall[:, blk, kt, oi],
                                xm[:, kt, cn:cn + wn],
                                start=(kt == 0),
                                stop=(kt == KT - 1),
                            )
                    for oi in range(OB):
                        ot = blk * OB + oi
                        nc.scalar.activation(
                            otile[:, oi, :wn],
                            ps[oi][:, :wn],
                            relu,
                            bias=btile[:, ot:ot + 1],
                        )
                        nc.vector.tensor_add(
                            otile[:, oi, :wn],
                            otile[:, oi, :wn],
                            xm[:, ot, cn:cn + wn],
                        )
                    nc.sync.dma_start(
                        out=out_r[:, blk * OB:(blk + 1) * OB, cn:cn + wn],
                        in_=otile[:, :, :wn],
                    )
    nc.compile()
    return nc


def kernel(x, cam_pred_ids, W, b, _want_results=False):
    x = np.ascontiguousarray(np.asarray(x), dtype=np.float32)
    W = np.asarray(W, dtype=np.float32)
    b = np.asarray(b, dtype=np.float32)
    ids = np.asarray(cam_pred_ids).astype(np.int64)

    counts = np.bincount(ids, minlength=NUM_EXPERTS)
    order = np.argsort(ids, kind="stable")
    np_tokens = max(512, int(counts.max()))

    # per-expert padded token index lists (pad with token 0; discarded later)
    starts = np.zeros(NUM_EXPERTS + 1, dtype=np.int64)
    np.cumsum(counts, out=starts[1:])
    idx = np.zeros((NUM_EXPERTS, np_tokens), dtype=np.int64)
    for e in range(NUM_EXPERTS):
        idx[e, : counts[e]] = order[starts[e] : starts[e + 1]]

    in_maps = []
    for e in range(NUM_EXPERTS):
        xg = x[idx[e]]  # [Np, DIM]
        xt = np.ascontiguousarray(xg.T, dtype=np.float16)  # [DIM, Np]
        # wt[blk, k, kt, obi, o] = W[e][(blk*OB+obi)*128 + o, kt*128 + k]
        wdev = np.ascontiguousarray(
            W[e].reshape(NBLK, OB, 128, KT, 128).transpose(0, 4, 3, 1, 2),
            dtype=np.float16,
        )
        in_maps.append({
            "wt": wdev,
            "xt": xt,
            "b": np.ascontiguousarray(b[e].reshape(OT, 128).T),
        })

    nc = _build_nc(np_tokens)
    res = run_bass_kernel_spmd(
        nc,
        in_maps,
        core_ids=list(range(NUM_EXPERTS)),
        trace=bool(int(os.environ.get("BASS_MOE_TRACE", "0"))),
    )

    out = np.empty_like(x)
    for e in range(NUM_EXPERTS):
        oute = res.results[e]["out"]  # [DIM, Np] f16
        valid = idx[e, : counts[e]]
        out[valid] = oute.T[: counts[e]].astype(np.float32)
    if _want_results:
        return out, res
    return out


# revision 9
# speedup vs baseline: 1.0929x; 1.0351x over previous
"""MoE routing kernel for Trainium2 (8 NeuronCores, expert-parallel).

out[i] = x[i] + relu(x[i] @ W[e].T + b[e]),  e = cam_pred_ids[i]

Strategy: route tokens by expert on the host, so core e computes ONLY
expert e's tokens with ONLY W[e] (8MB in f16 instead of 128MB).  The
device computes hT[o, n] = sum_k WT[k, o] * xT[k, n] with K on
partitions for both operands; everything on device is f16 except PSUM
and the bias (end-to-end rel err ~3e-4).

Loop structure (v3): work is [column chunk <=512] x [ot group].  The
FIRST group spans ot 0-7 x chunk 0 on all 8 PSUM banks with the kt
loop outermost: its first matmul needs only the first k-tile of x and
of the weight stream, and its steady HBM demand is ~225 GB/s (weights
152 + x 76), inside the ~315 GB/s per-core budget -- so the PE starts
~1.5us in and never starves.  Remaining groups are 4 ot x 1 chunk
(ping-ponging 4-bank PSUM sets) ordered so a fresh weight stream and
a fresh x chunk are never needed in the same window.  The whole
weight stack stays resident in SBUF (64KB/partition); x is loaded
once (f16, doubles as matmul rhs and residual).  Output is stored f16
and widened on the host.

Ring split (HWDGE rings exist only on SP/sync and Activation/scalar):
sync = weight stream (half 0 in 2-kt pieces to race the PE) + output
stores (first needed ~28us in, after all weight triggers); scalar =
x (chunk 0 in 4-kt pieces) + bias.
"""

import os
import numpy as np

import concourse.bass as bass
from concourse import bacc
import concourse.mybir as mybir
import concourse.tile as tile
from concourse.bass_utils import run_bass_kernel_spmd

MODE = "f16"  # informational; device math is f16 (PSUM f32)

NUM_EXPERTS = 8
DIM = 2048
KT = DIM // 128   # 16 k-tiles
OT = DIM // 128   # 16 o-tiles
HB = 8            # o-tiles per half (first super-group spans one half)
OB = 4            # o-tiles per regular group (4 PSUM banks)


def _chunks(np_tokens: int) -> list[tuple[int, int]]:
    """Split the free dim into matmul chunks of <=512 (one PSUM bank),
    each >=232 when possible (below ~230 cols the 97ns LDWEIGHTS shadow
    outruns the matmul and the PE stalls on weight loads)."""
    out = []
    pos = 0
    rem = np_tokens
    while rem > 0:
        if rem > 512 + 232:
            take = 512
        elif rem > 512:
            take = (rem + 1) // 2  # two chunks, both >=232
        else:
            take = rem
        out.append((pos, take))
        pos += take
        rem -= take
    return out


def _build_nc(np_tokens: int):
    f32 = mybir.dt.float32
    f16 = mybir.dt.float16

    nc = bacc.Bacc()
    # wt[h, k, kt, j, o] = W[(h*8+j)*128 + o, kt*128 + k]: per partition k,
    # one half is a contiguous 16KB run in consumption (kt-major) order.
    wt_d = nc.declare_dram_parameter("wt", [2, 128, KT, HB, 128], f16,
                                     isOutput=False)
    xt_d = nc.declare_dram_parameter("xt", [DIM, np_tokens], f16, isOutput=False)
    b_d = nc.declare_dram_parameter("b", [128, OT], f32, isOutput=False)
    out_d = nc.declare_dram_parameter("out", [DIM, np_tokens], f16, isOutput=True)

    chunks = _chunks(np_tokens)
    relu = mybir.ActivationFunctionType.Relu

    xt_r = xt_d.rearrange("(t p) n -> p t n", p=128)
    out_r = out_d.rearrange("(t p) n -> p t n", p=128)

    with tile.TileContext(nc) as tc:
        with (
            tc.tile_pool(name="wp", bufs=1) as wp,
            tc.tile_pool(name="xp", bufs=1) as xp,
            tc.tile_pool(name="bp", bufs=1) as bp,
            tc.tile_pool(name="op", bufs=3) as op,
            tc.tile_pool(name="pp", bufs=1, space="PSUM") as pp,
        ):
            wall = wp.tile([128, 2, KT, HB, 128], f16, name="wall")
            xm = xp.tile([128, KT, np_tokens], f16, name="xm")
            btile = bp.tile([128, OT], f32, name="btile")

            # Half 0 of the weights in 2-kt pieces so the super-group's
            # matmuls only wait for the head of the stream; half 1 in two
            # transfers (first consumed ~56us in, long after it lands).
            for s in range(0, KT, 2):
                nc.sync.dma_start(out=wall[:, 0, s:s + 2],
                                  in_=wt_d[0, :, s:s + 2])
            nc.sync.dma_start(out=wall[:, 1, :KT // 2], in_=wt_d[1, :, :KT // 2])
            nc.sync.dma_start(out=wall[:, 1, KT // 2:], in_=wt_d[1, :, KT // 2:])

            # x chunk 0 in 4-kt pieces (consumed one k-tile per 1.7us in the
            # super-group); later chunks whole.
            (c0, w0) = chunks[0]
            for s in range(0, KT, 4):
                nc.scalar.dma_start(out=xm[:, s:s + 4, c0:c0 + w0],
                                    in_=xt_r[:, s:s + 4, c0:c0 + w0])
            nc.scalar.dma_start(out=btile, in_=b_d[:, :])
            for (cn, wn) in chunks[1:]:
                nc.scalar.dma_start(out=xm[:, :, cn:cn + wn],
                                    in_=xt_r[:, :, cn:cn + wn])

            def drain(ps_list, ots, cn, wn):
                otile = op.tile([128, len(ots), 512], f16, name="otile", tag="o")
                for oi, ot in enumerate(ots):
                    nc.scalar.activation(
                        otile[:, oi, :wn],
                        ps_list[oi][:, :wn],
                        relu,
                        bias=btile[:, ot:ot + 1],
                    )
                    nc.vector.tensor_add(
                        otile[:, oi, :wn],
                        otile[:, oi, :wn],
                        xm[:, ot, cn:cn + wn],
                    )
                nc.sync.dma_start(
                    out=out_r[:, ots[0]:ots[0] + len(ots), cn:cn + wn],
                    in_=otile[:, :, :wn],
                )

            def mm_group(h, js, cn, wn, ps_list):
                for kt in range(KT):
                    for oi, j in enumerate(js):
                        nc.tensor.matmul(
                            ps_list[oi][:, :wn],
                            wall[:, h, kt, j],
                            xm[:, kt, cn:cn + wn],
                            start=(kt == 0),
                            stop=(kt == KT - 1),
                        )

            # Super-group: ot 0-7 x chunk 0 on all 8 banks.
            (c0, w0) = chunks[0]
            ps8 = [pp.tile([128, 512], f32, name="ps", tag=f"ps{oi}")
                   for oi in range(HB)]
            mm_group(0, range(HB), c0, w0, ps8)
            drain(ps8[:OB], list(range(0, OB)), c0, w0)
            drain(ps8[OB:], list(range(OB, HB)), c0, w0)

            # Remaining groups: 4 ot x 1 chunk; for each half, walk the
            # leftover (ot-block, chunk) pairs so fresh weights (next half)
            # and fresh x (later chunks) are never demanded together.
            rest = []
            for bj in (0, 1):          # ot blocks within half 0
                for ci, (cn, wn) in enumerate(chunks):
                    if ci == 0:
                        continue       # covered by the super-group
                    rest.append((0, bj, cn, wn))
            for bj in (0, 1):          # half 1: all chunks
                for (cn, wn) in chunks:
                    rest.append((1, bj, cn, wn))

            # 8 physical banks = 8 bufs=1 tags; regular groups ping-pong
            # between tag sets 0-3 and 4-7 (allocating a tag again waits for
            # its previous group's activation, i.e. the bank is drained).
            for gi, (h, bj, cn, wn) in enumerate(rest):
                js = range(bj * OB, (bj + 1) * OB)
                t0 = (gi % 2) * OB
                ps4 = [pp.tile([128, 512], f32, name="ps", tag=f"ps{t0 + oi}")
                       for oi in range(OB)]
                mm_group(h, js, cn, wn, ps4)
                drain(ps4, [h * HB + j for j in js], cn, wn)
    nc.compile()
    return nc


def kernel(x, cam_pred_ids, W, b, _want_results=False):
    x = np.ascontiguousarray(np.asarray(x), dtype=np.float32)
    W = np.asarray(W, dtype=np.float32)
    b = np.asarray(b, dtype=np.float32)
    ids = np.asarray(cam_pred_ids).astype(np.int64)

    counts = np.bincount(ids, minlength=NUM_EXPERTS)
    order = np.argsort(ids, kind="stable")
    np_tokens = max(512, int(counts.max()))

    # per-expert padded token index lists (pad with token 0; discarded later)
    starts = np.zeros(NUM_EXPERTS + 1, dtype=np.int64)
    np.cumsum(counts, out=starts[1:])
    idx = np.zeros((NUM_EXPERTS, np_tokens), dtype=np.int64)
    for e in range(NUM_EXPERTS):
        idx[e, : counts[e]] = order[starts[e] : starts[e + 1]]

    in_maps = []
    for e in range(NUM_EXPERTS):
        xg = x[idx[e]]  # [Np, DIM]
        xt = np.ascontiguousarray(xg.T, dtype=np.float16)  # [DIM, Np]
        # wt[h, k, kt, j, o] = W[e][(h*8+j)*128 + o, kt*128 + k]
        wdev = np.ascontiguousarray(
            W[e].reshape(2, HB, 128, KT, 128).transpose(0, 4, 3, 1, 2),
            dtype=np.float16,
        )
        in_maps.append({
            "wt": wdev,
            "xt": xt,
            "b": np.ascontiguousarray(b[e].reshape(OT, 128).T),
        })

    nc = _build_nc(np_tokens)
    res = run_bass_kernel_spmd(
        nc,
        in_maps,
        core_ids=list(range(NUM_EXPERTS)),
        trace=bool(int(os.environ.get("BASS_MOE_TRACE", "0"))),
    )

    out = np.empty_like(x)
    for e in range(NUM_EXPERTS):
        oute = res.results[e]["out"]  # [DIM, Np] f16
        valid = idx[e, : counts[e]]
        out[valid] = oute.T[: counts[e]].astype(np.float32)
    if _want_results:
        return out, res
    return out
